# revision 24
# baseline (speedup 1.0000x reference)
"""PoPE attention kernel for Trainium2, sharded over 8 NeuronCores by heads.

Problem: B=1, S=2048, DIM=1024, H=16 heads, D=64.
  q/k/v = x @ w{q,k,v}^T ; PoPE embed (softplus magnitude x cos/sin phase);
  scores = q_emb @ k_emb^T / sqrt(D); softmax; out = attn @ v; y = out @ wo^T.

Sharding: 2 heads per core. Each core computes its heads' projections,
attention, and a partial output projection (its 128 channels of wo);
host sums the 8 partial y's (f32) - no on-chip collectives.

Layouts are "transposed" (feature-major): xT [DIM, S] so that every
matmul has its contraction on the partition axis with no on-chip
transposes. All matmuls are bf16 with f32 PSUM accumulate.

The kernel is ACT(exp)-bound: softmax needs 2 heads x 2048 x 2048 exps
per core = 8.4M elements at 1 elem/cycle/lane = ~55us minimum on the
scalar engine. Everything else is scheduled to (a) start that exp stream
as early as possible and (b) never let it stall:
 - One ACT table load (natural_log_exp_and_others covers Exp+Ln+Copy).
 - xt and the trig tables stream in 512-token blocks; q/k projections,
   softplus and embeds pipeline per block, so the first score chunk
   needs only block 0+1 (the first query superblock / first key chunks).
 - The second half's softplus/ln ACTIVATEs are issued interleaved into
   the score-exp stream (no idle, no extra table loads).
 - v-projection fills PE gaps during stage 0 on the freed psm banks.
The softmax skips max-subtraction (scores/8 are bounded ~4.3), and the
rowsum comes free from a ones-column appended to v in the attn@v matmul.
The 1/rowsum is a DMA spread over 128 partitions + DVE reciprocal; the
final normalize broadcasts it with a ones-stationary matmul (PE is idle
there, gpsimd dispatch+drain is ~3.5us) so the output projection starts
~5us after the last exp.
"""
import math

import numpy as np
import ml_dtypes

import concourse.bacc as bacc
import concourse.mybir as mybir
from concourse import tile
from concourse.bass_utils import run_bass_kernel_spmd

BF16 = ml_dtypes.bfloat16
FP8 = ml_dtypes.float8_e4m3
W_SCALE = 32.0
S, DIM, H, D = 2048, 1024, 16, 64
NCORES = 8
HPC = H // NCORES          # heads per core = 2
ED = 2 * D                 # embedding width per head = 128
KI = DIM // 128            # contraction chunks for projections = 8
KC = S // 128              # key-token chunks = 16
QC = S // 512              # 512-token blocks = 4
OC = DIM // 128            # output-channel chunks = 8

NLE_SET = 6  # index of natural_log_exp_and_others in get_activation_tables

_compiled_nc = None


def _build_body(nc, tc, persist, ps_pool, out_pool, xt_pool, exp_pool, ext):
    dt = mybir.dt
    AF = mybir.ActivationFunctionType
    xt_ext, w_ext, tq_ext, tk_ext, wo_ext, y_ext = ext
    QH = 1024                  # query superblock width

    # Single ACT table set for the whole kernel (Exp+Ln+Copy): loads are
    # ~1.3-2.7us each and the default pass would use three of them.
    nc.scalar.add_instruction(mybir.InstLoadActFuncSet(
        name=f"I-{nc.next_id()}", act_func_set_id=NLE_SET, ins=[], outs=[]))

    # ---- HAM warmup: dummy matmuls on junk data while the input DMAs run,
    # so the PE clock-gate reaches 2.4 GHz before the real matmuls start ----
    warm_sb = persist.tile([128, 512], dt.bfloat16)
    nc.vector.memset(warm_sb[:], 0.0)
    warm_ps = ps_pool.tile([128, 512], dt.float32, name="warm_ps", tag="scA")
    for i in range(16):
        nc.tensor.matmul(warm_ps[:], warm_sb[:, 0:128], warm_sb[:],
                         start=(i == 0), stop=(i == 15))

    # ---- phase A: input DMAs, 512-token-block pipelined ----
    w_sb = persist.tile([128, 3, KI, ED], dt.bfloat16)
    nc.sync.dma_start(w_sb[:], w_ext[:])
    xt = xt_pool.tile([128, QC, KI, 512], dt.bfloat16)
    tq_sb = persist.tile([128, QC, 2, 512], dt.bfloat16)
    tk_sb = persist.tile([128, QC, 2, 512], dt.bfloat16)
    for qc in range(QC):
        for kh in range(2):
            nc.sync.dma_start(xt[:, qc, 4 * kh:4 * kh + 4], xt_ext[:, qc, 4 * kh:4 * kh + 4])
        nc.sync.dma_start(tq_sb[:, qc], tq_ext[:, qc])
        nc.sync.dma_start(tk_sb[:, qc], tk_ext[:, qc])
    wo_sb = persist.tile([128, DIM], dt.bfloat16)
    nc.sync.dma_start(wo_sb[:], wo_ext[:])
    # v with a ones column appended per (head, key chunk)
    v_sb = persist.tile([128, HPC, KC, D + 1], dt.bfloat16)
    nc.gpsimd.memset(v_sb[:, 0, :, D], 1.0)
    nc.gpsimd.memset(v_sb[:, 1, :, D], 1.0)
    ones_sb = persist.tile([1, 64], dt.bfloat16)
    nc.gpsimd.memset(ones_sb[:], 1.0)
    emb_q = [persist.tile([128, S], dt.bfloat16, name=f"embq{h}", tag=f"embq{h}")
             for h in range(HPC)]
    emb_k = [persist.tile([128, S], dt.bfloat16, name=f"embk{h}", tag=f"embk{h}")
             for h in range(HPC)]
    outT = persist.tile([128, S], dt.bfloat16)

    # PSUM: four 2-bank tags. psm(q,lo0)->scA, psm(k,lo0)->scB free early
    # for the score double-buffer; psm(*,lo1) sit on avA/avB which are not
    # needed until the attn@v stages.
    PSM_TAG = {(0, 0): "scA", (1, 0): "scB", (0, 1): "avA", (1, 1): "avB"}
    psm = {}
    for p in range(2):
        for lo in range(2):
            psm[(p, lo)] = ps_pool.tile([128, QH], dt.float32,
                                        name=f"psm{p}_{lo}", tag=PSM_TAG[(p, lo)])
    # ---- phase B: q/k projections. Per half: q's two blocks first, then
    # k's, so q's softplus/ln/embeds (ACT+DVE) overlap k's matmuls ----
    def proj(p, lo):
        for b in range(2):
            for ki in range(KI):
                nc.tensor.matmul(
                    psm[(p, lo)][:, b * 512:b * 512 + 512],
                    w_sb[:, p, ki, :],
                    xt[:, 2 * lo + b, ki, :],
                    start=(ki == 0), stop=(ki == KI - 1),
                )

    proj(0, 0)
    proj(1, 0)
    proj(0, 1)
    proj(1, 1)

    # softplus(x) = ln(1 + e^x). tmp/mag column layout: [p, lo*QH:...]
    tmp = xt_pool.tile([128, 2, S], dt.float32, name="sp", tag="sp")
    mag = xt_pool.tile([128, 2, S], dt.bfloat16, name="mag", tag="mag")
    qk_mag = [mag[:, 0, :], mag[:, 1, :]]

    def sp_exp(p, lo):
        nc.scalar.activation(tmp[:, p, lo * QH:(lo + 1) * QH],
                             psm[(p, lo)][:], AF.Exp)

    def sp_ln(p, lo):
        nc.scalar.activation(mag[:, p, lo * QH:(lo + 1) * QH],
                             tmp[:, p, lo * QH:(lo + 1) * QH], AF.Ln, bias=1.0)

    def embeds(lo, h, p):
        emb = (emb_q, emb_k)[p]
        trig = (tq_sb, tk_sb)[p]
        r = slice(64 * h, 64 * h + 64)
        for t in range(2):  # 0=cos part, 1=sin part
            e = slice(64 * t, 64 * t + 64)
            for b in range(2):
                cb = slice(lo * QH + b * 512, lo * QH + b * 512 + 512)
                nc.vector.tensor_mul(emb[h][e, cb], qk_mag[p][r, cb],
                                     trig[r, 2 * lo + b, t, :])

    # low-half softplus + embeds: this is the critical path to the first
    # score chunk (needs emb_q[0] cols 0:1024 and emb_k[0] cols 0:128+)
    sp_exp(0, 0)
    sp_ln(0, 0)
    sp_exp(1, 0)
    sp_ln(1, 0)
    embeds(0, 0, 0)
    embeds(0, 0, 1)
    embeds(0, 1, 0)
    embeds(0, 1, 1)

    # ---- phase C: v projection (token-major directly), issued interleaved
    # into stage 0 below so its matmuls fill the ACT-paced PE gaps ----
    def v_group(g, tag):
        psv = ps_pool.tile([128, 4, 128], dt.float32, name=f"psv{g}", tag=tag)
        for sub in range(4):
            t = 4 * g + sub
            for ki in range(KI):
                nc.tensor.matmul(
                    psv[:, sub, :],
                    xt[:, t // 4, ki, (t % 4) * 128:(t % 4) * 128 + 128],
                    w_sb[:, 2, ki, :],
                    start=(ki == 0), stop=(ki == KI - 1),
                )
        for h in range(HPC):
            for sub in range(4):
                t = 4 * g + sub
                nc.vector.tensor_copy(
                    v_sb[:, h, t, 0:D], psv[:, sub, 64 * h:64 * h + 64])

    # ---- phase D: attention in two query superblocks of 1024; inside each,
    # (head, key-half) stages. Score tiles double-buffer on scA/scB. ----
    av_ps = {}
    exp_tiles = {}

    def scores_chunk(h, kc, qh):
        e = exp_pool.tile([128, QH], dt.bfloat16,
                          name=f"exp{qh}_{h}_{kc}", tag=f"exp{qh}_{kc % 8}")
        exp_tiles[(h, kc, qh)] = e
        sc = ps_pool.tile([128, QH], dt.float32, name=f"sc{qh}_{h}_{kc}",
                          tag=("scA", "scB")[kc % 2])
        for q2 in range(2):
            nc.tensor.matmul(
                sc[:, q2 * 512:(q2 + 1) * 512],
                emb_k[h][:, kc * 128:(kc + 1) * 128],
                emb_q[h][:, qh * QH + q2 * 512:qh * QH + (q2 + 1) * 512],
                start=True, stop=True,
            )
        nc.scalar.activation(e[:], sc[:], AF.Exp, scale=1.0 / math.sqrt(D))

    def av_chunk(h, kc, qh):
        pav = av_ps[(h, qh)]
        e = exp_tiles[(h, kc, qh)]
        for q2 in range(2):
            nc.tensor.matmul(
                pav[0:D + 1, q2 * 512:(q2 + 1) * 512],
                v_sb[:, h, kc, :],
                e[:, q2 * 512:(q2 + 1) * 512],
                start=(kc == 0), stop=(kc == KC - 1),
            )

    def normalize(h, qh, final=False):
        # rowsum row (row D) evicted first so the 1/rowsum chain (DMA
        # spread across 128 partitions -> DVE reciprocal -> DMA back ->
        # broadcast) starts immediately; acopy runs in parallel.
        pav = av_ps[(h, qh)]
        if final:
            # column-halved, fully pipelined: the chain's 5 cross-engine
            # hops cost ~0.5-1us latency each, so the first outT half (and
            # with it the first output-projection matmuls) lands ~3.5us
            # after the last exp instead of ~7. PE broadcasts 1/rowsum via
            # a ones-stationary matmul (it is idle here; gpsimd dispatch +
            # drain is ~3.5us).
            rcopy = persist.tile([1, QH], dt.float32, name=f"rc{h}_{qh}",
                                 tag=f"rc{h}")
            acopy = persist.tile([D, QH], dt.float32,
                                 name=f"acopy{h}_{qh}", tag=f"acopy{h}")
            rs128 = persist.tile([128, QH // 128], dt.float32,
                                 name=f"rs128_{h}_{qh}", tag=f"rs128_{h}")
            rr128 = persist.tile([128, QH // 128], dt.bfloat16,
                                 name=f"rr128_{h}_{qh}", tag=f"rr128_{h}")
            rr = persist.tile([1, QH], dt.bfloat16, name=f"rr{h}_{qh}",
                              tag=f"rr{h}")
            rsb_ps = ps_pool.tile([64, QH], dt.float32, name="rsb_ps", tag="scA")
            NH = QH // 2
            CS = [slice(hf * NH, (hf + 1) * NH) for hf in range(2)]
            SS = [slice(hf * (QH // 256), (hf + 1) * (QH // 256)) for hf in range(2)]
            for hf in range(2):
                nc.scalar.activation(rcopy[0:1, CS[hf]], pav[D:D + 1, CS[hf]],
                                     AF.Copy)
                nc.sync.dma_start(rs128[:, SS[hf]], rcopy[0:1, CS[hf]])
            for hf in range(2):
                nc.vector.tensor_copy(acopy[:, CS[hf]], pav[0:D, CS[hf]])
            with nc.allow_low_precision(reason="softmax 1/rowsum bf16"):
                for hf in range(2):
                    nc.vector.reciprocal(rr128[:, SS[hf]], rs128[:, SS[hf]])
            for hf in range(2):
                nc.sync.dma_start(rr[0:1, CS[hf]], rr128[:, SS[hf]])
                nc.tensor.matmul(rsb_ps[:, CS[hf]], ones_sb[:], rr[0:1, CS[hf]],
                                 start=True, stop=True)
            for hf in range(2):
                nc.vector.tensor_mul(
                    outT[64 * h:64 * h + 64,
                         qh * QH + hf * NH:qh * QH + (hf + 1) * NH],
                    acopy[:, CS[hf]], rsb_ps[:, CS[hf]])
            return
        rcopy = persist.tile([1, QH], dt.float32,
                             name=f"rc{h}_{qh}", tag=f"rc{h}")
        nc.vector.tensor_copy(rcopy[:], pav[D:D + 1, :])
        # acopy right behind rcopy on DVE so the PSUM bank frees before the
        # DMA-roundtrip-gated reciprocal
        acopy = persist.tile([D, QH], dt.float32,
                             name=f"acopy{h}_{qh}", tag=f"acopy{h}")
        nc.vector.tensor_copy(acopy[:], pav[0:D, :])
        rs128 = persist.tile([128, QH // 128], dt.float32,
                             name=f"rs128_{h}_{qh}", tag=f"rs128_{h}")
        nc.sync.dma_start(rs128[:], rcopy[:])
        rr128 = persist.tile([128, QH // 128], dt.bfloat16,
                             name=f"rr128_{h}_{qh}", tag=f"rr128_{h}")
        with nc.allow_low_precision(reason="softmax 1/rowsum in bf16 is ~0.4% scale noise"):
            nc.vector.reciprocal(rr128[:], rs128[:])
        rr = persist.tile([1, QH], dt.bfloat16, name=f"rr{h}_{qh}", tag=f"rr{h}")
        nc.sync.dma_start(rr[:], rr128[:])
        rsb_t = persist.tile([64, QH], dt.bfloat16,
                             name=f"rsb{h}_{qh}", tag=f"rsb{h}")
        nc.gpsimd.partition_broadcast(rsb_t[:], rr[:])
        nc.vector.tensor_mul(outT[64 * h:64 * h + 64, qh * QH:(qh + 1) * QH],
                             acopy[:], rsb_t[:])

    def oproj(qh, oc, tags=("avA", "avB"), act_ok=False):
        c = slice(qh * QH, (qh + 1) * QH)
        psy = ps_pool.tile([128, QH], dt.float32, name=f"psy{qh}_{oc}",
                           tag=tags[oc % len(tags)])
        for q2 in range(2):
            nc.tensor.matmul(
                psy[:, q2 * 512:(q2 + 1) * 512],
                wo_sb[:, oc * 128:(oc + 1) * 128],
                outT[:, qh * QH + q2 * 512:qh * QH + (q2 + 1) * 512],
                start=True, stop=True,
            )
        y_sb = out_pool.tile([128, QH], dt.bfloat16, name=f"y{qh}_{oc}", tag="y")
        if act_ok:
            # ACT is idle here: split the eviction across ACT and DVE halves
            nc.scalar.activation(y_sb[:, 0:QH // 2], psy[:, 0:QH // 2], AF.Copy)
            nc.vector.tensor_copy(y_sb[:, QH // 2:QH], psy[:, QH // 2:QH])
        else:
            # ACT is saturated with exps: keep the eviction off it
            nc.vector.tensor_copy(y_sb[:], psy[:])
        nc.sync.dma_start(y_ext[oc, :, c], y_sb[:])

    LAG = 1
    for qh in range(2):
        # stage 0: scores/exp (h0, 1st key half). Fillers keep PE dense:
        # superblock 0 interleaves the 2nd-half softplus (on ACT) and the
        # v-projection groups; superblock 1 runs superblock 0's output
        # projection on the freed avA/avB banks.
        for j in range(8):
            if qh == 1:
                oproj(0, j)
            scores_chunk(0, j, qh)
            if qh == 0:
                # 2nd-half softplus interleaves into the exp stream; its
                # Exps free avA/avB for the v-projection groups below
                if j == 0:
                    sp_exp(0, 1)
                    sp_ln(0, 1)
                    v_group(0, "avA")
                elif j == 1:
                    sp_exp(1, 1)
                    sp_ln(1, 1)
                    v_group(1, "avB")
                elif j == 2:
                    embeds(1, 0, 0)
                    embeds(1, 0, 1)
                    v_group(2, "avA")
                elif j == 3:
                    embeds(1, 1, 0)
                    embeds(1, 1, 1)
                    v_group(3, "avB")
        # stage 1: scores/exp (h0, 2nd half) + av (h0, 1st half)
        av_ps[(0, qh)] = ps_pool.tile([128, QH], dt.float32,
                                      name=f"av0_{qh}", tag="avA")
        for j in range(8):
            av_chunk(0, j, qh)
            scores_chunk(0, 8 + j, qh)
        # stage 2: scores/exp (h1, 1st half) + av (h0, 2nd half)
        av_ps[(1, qh)] = ps_pool.tile([128, QH], dt.float32,
                                      name=f"av1_{qh}", tag="avB")
        for j in range(8):
            av_chunk(0, 8 + j, qh)
            scores_chunk(1, j, qh)
        normalize(0, qh)
        # stage 3: scores/exp (h1, 2nd half) + av (h1, 1st half)
        for j in range(8):
            if j >= LAG:
                av_chunk(1, j - LAG, qh)
            scores_chunk(1, 8 + j, qh)
        for kc in range(8 - LAG, KC):
            av_chunk(1, kc, qh)
        if qh == 1:
            # keep the PE busy through the 1/rowsum chain so HAM does not
            # drop the clock to half duty for the output projection
            junk = ps_pool.tile([128, 512], dt.float32, name="junk", tag="scB")
            for i in range(20):
                nc.tensor.matmul(junk[:], warm_sb[:, 0:128], warm_sb[:],
                                 start=(i == 0), stop=(i == 19))
        normalize(1, qh, final=(qh == 1))

    # ---- phase E: output projection for the last superblock (scA/scB are
    # free after the last exp -> deep psy pipeline) ----
    for oc in range(OC):
        oproj(1, oc, tags=("avB", "scB", "avA"), act_ok=True)


def _build():
    nc = bacc.Bacc()
    dt = mybir.dt

    ext = (
        nc.declare_dram_parameter("xt", [128, QC, KI, 512], dt.bfloat16, isOutput=False),
        nc.declare_dram_parameter("w", [128, 3, KI, ED], dt.bfloat16, isOutput=False),
        nc.declare_dram_parameter("trig_q", [128, QC, 2, 512], dt.bfloat16, isOutput=False),
        nc.declare_dram_parameter("trig_k", [128, QC, 2, 512], dt.bfloat16, isOutput=False),
        nc.declare_dram_parameter("woT", [128, DIM], dt.bfloat16, isOutput=False),
        nc.declare_dram_parameter("yT", [OC, 128, S], dt.bfloat16, isOutput=True),
    )

    with tile.TileContext(nc) as tc:
        with tc.tile_pool(name="persist", bufs=1) as persist, \
             tc.tile_pool(name="ps", bufs=1, space="PSUM") as ps_pool, \
             tc.tile_pool(name="out", bufs=4) as out_pool, \
             tc.tile_pool(name="xtp", bufs=1) as xt_pool, \
             tc.tile_pool(name="expp", bufs=2) as exp_pool:
            _build_body(nc, tc, persist, ps_pool, out_pool, xt_pool, exp_pool, ext)
    nc.compile()
    return nc


def _get_nc():
    global _compiled_nc
    if _compiled_nc is None:
        _compiled_nc = _build()
    return _compiled_nc


def _prep_inputs(x, wq, wk, wv, wo, pope_bias):
    """Host-side sharding + layout prep. Returns in_maps for the 8 cores."""
    x2 = np.ascontiguousarray(x.reshape(S, DIM).astype(np.float32))

    # trig tables (f64 phases for accuracy)
    inv = 10000.0 ** (-(np.arange(D, dtype=np.float64) / D))
    pos = np.arange(S, dtype=np.float64)
    freqs = pos[:, None] * inv[None, :]                       # [S, D]
    bias = np.clip(pope_bias.astype(np.float64), -2 * math.pi, 0.0)  # [H, D]

    cos_q = np.cos(freqs).T.astype(BF16)                      # [D, S]
    sin_q = np.sin(freqs).T.astype(BF16)
    trig_q = np.empty((128, 2, S), BF16)
    trig_q[0:64, 0] = cos_q
    trig_q[64:128, 0] = cos_q
    trig_q[0:64, 1] = sin_q
    trig_q[64:128, 1] = sin_q
    # -> blocked [128, QC, 2, 512]
    trig_qb = np.ascontiguousarray(
        trig_q.reshape(128, 2, QC, 512).transpose(0, 2, 1, 3))

    # xt[q, qc, ki, c] = x[qc*512 + c, ki*128 + q]
    xt = np.ascontiguousarray(
        x2.T.reshape(KI, 128, QC, 512).transpose(1, 2, 0, 3)).astype(BF16)

    in_maps = []
    for c in range(NCORES):
        hs = slice(c * HPC * D, (c + 1) * HPC * D)            # head-channel slice
        # lhsT chunk for proj p is w_p[hs].T[ki*128:(ki+1)*128, :]
        w = np.empty((128, 3, KI, ED), BF16)
        for p, wm in enumerate((wq, wk, wv)):
            wt = np.ascontiguousarray(wm[hs, :].astype(np.float32).T)  # [DIM, ED]
            w[:, p] = wt.reshape(KI, 128, ED).transpose(1, 0, 2)

        ph = freqs[None, :, :] + bias[c * HPC:(c + 1) * HPC, None, :]  # [HPC, S, D]
        trig_k = np.empty((128, 2, S), BF16)
        for h in range(HPC):
            trig_k[64 * h:64 * h + 64, 0] = np.cos(ph[h]).T
            trig_k[64 * h:64 * h + 64, 1] = np.sin(ph[h]).T
        trig_kb = np.ascontiguousarray(
            trig_k.reshape(128, 2, QC, 512).transpose(0, 2, 1, 3))

        woT = np.ascontiguousarray(wo[:, hs].astype(np.float32).T).astype(BF16)

        in_maps.append({
            "xt": xt, "w": w, "trig_q": trig_qb, "trig_k": trig_kb, "woT": woT,
        })
    return in_maps


def kernel(x, wq, wk, wv, wo, pope_bias):
    nc = _get_nc()
    in_maps = _prep_inputs(np.asarray(x), np.asarray(wq), np.asarray(wk),
                           np.asarray(wv), np.asarray(wo), np.asarray(pope_bias))
    res = run_bass_kernel_spmd(nc, in_maps, list(range(NCORES)))
    y = np.zeros((DIM, S), np.float32)
    for c in range(NCORES):
        y += res.results[c]["yT"].reshape(DIM, S).astype(np.float32)
    return np.ascontiguousarray(y.T).reshape(1, S, DIM)


if __name__ == "__main__":
    rng = np.random.default_rng(0)
    out = kernel(
        x=rng.standard_normal((1, S, DIM)).astype(np.float32),
        wq=rng.standard_normal((DIM, DIM)).astype(np.float32) / 32,
        wk=rng.standard_normal((DIM, DIM)).astype(np.float32) / 32,
        wv=rng.standard_normal((DIM, DIM)).astype(np.float32) / 32,
        wo=rng.standard_normal((DIM, DIM)).astype(np.float32) / 32,
        pope_bias=-rng.random((H, D), np.float32) * 3.0,
    )
    print("out", out.shape, out.dtype, np.abs(out).mean())
